# revision 5
# baseline (speedup 1.0000x reference)
"""Trainium2 Bass kernel: single-head attention with QKV projections.

Problem (hardcoded): q/k/v [4,2048,1024] fp32, W_q/W_k/W_v [1024,1024] fp32;
out = softmax((x@Wq^T)(x@Wk^T)^T/32) @ (x@Wv^T), fp32 [4,2048,1024].

Sharding: 8 cores = 4 batches x 2 query-halves, pair-collective K/V
exchange with a permutation-invariant key layout.

v3:
- S = Q^T K in fp8-e4m3 DoubleRow (256-deep contraction per instr,
  measured 216ns/MM): W_q/W_k scaled by 32 on host so Q,K fill e4m3's
  range; exp() folds 1/32768 back in.  V/attention path stays bf16
  (CPU-exact sim: rel err 1.77e-2 vs the 2e-2 gate; any fp8 on the V
  path or projections busts the budget).
- Startup: wk/kin DMAs split into halves and the K-projection sweeps
  dt-outer over open PSUM groups, so the PE streams while the inputs
  arrive instead of waiting for the full 4MB (first group needs 1.25MB).
- All PSUM accumulators are [128,512] one-bank tiles (pool bufs=7 +
  1 ssum bank) to loosen the bank-reuse stalls.
- psum->SBUF copies alternate Vector/Scalar; output tail split in 4.
"""

import numpy as np
import ml_dtypes

P = 128
D = 1024
E = 1024
QL = 1024
KL = 2048
KH = 1024
DT, ET, QT, KT = D // P, E // P, QL // P, KL // P
KHT = KH // P

_CACHE = {}


def _build_nc():
    from contextlib import ExitStack

    import concourse.bass as bass
    import concourse.mybir as mybir
    import concourse.tile as tile
    from concourse import bacc

    BF = mybir.dt.bfloat16
    F8 = mybir.dt.float8e4
    F32 = mybir.dt.float32
    AFT = mybir.ActivationFunctionType
    DR = mybir.MatmulPerfMode.DoubleRow

    nc = bacc.Bacc("TRN2", target_bir_lowering=False, debug=False,
                   enable_asserts=False, num_devices=8)

    qinT = nc.dram_tensor("qinT", [D, QL], BF, kind="ExternalInput").ap()
    kinT = nc.dram_tensor("kinT", [D, KH], BF, kind="ExternalInput").ap()
    vinT = nc.dram_tensor("vinT", [D, KH], BF, kind="ExternalInput").ap()
    wqT = nc.dram_tensor("wqT", [D, E], BF, kind="ExternalInput").ap()
    wkT = nc.dram_tensor("wkT", [D, E], BF, kind="ExternalInput").ap()
    wvT = nc.dram_tensor("wvT", [D, E], BF, kind="ExternalInput").ap()
    out = nc.dram_tensor("out", [QL, E], F32, kind="ExternalOutput").ap()

    RG = [[0, 1], [2, 3], [4, 5], [6, 7]]

    with tile.TileContext(nc) as tc, ExitStack() as ctx:
        wpool = ctx.enter_context(tc.tile_pool(name="w", bufs=2))
        apool = ctx.enter_context(tc.tile_pool(name="acts", bufs=2))
        qt_pool = ctx.enter_context(tc.tile_pool(name="qT", bufs=1))
        kt_pool = ctx.enter_context(tc.tile_pool(name="kT", bufs=1))
        v_pool = ctx.enter_context(tc.tile_pool(name="V", bufs=1))
        pt_pool = ctx.enter_context(tc.tile_pool(name="pT", bufs=1))
        o_pool = ctx.enter_context(tc.tile_pool(name="o", bufs=3))
        small = ctx.enter_context(tc.tile_pool(name="small", bufs=1))
        r_pool = ctx.enter_context(tc.tile_pool(name="r", bufs=2))
        ps = ctx.enter_context(tc.tile_pool(name="ps", bufs=7, space="PSUM"))
        ps_s = ctx.enter_context(tc.tile_pool(name="ps_s", bufs=1, space="PSUM"))
        dram = ctx.enter_context(tc.tile_pool(name="dram", bufs=1, space="DRAM"))

        ones_t = small.tile([P, 1], BF, tag="ones")
        nc.vector.memset(ones_t, 1.0)

        qT_sb = qt_pool.tile([P, ET, QL], F8, tag="qT")
        kT_sb = kt_pool.tile([P, ET, KL], F8, tag="kT")
        V_sb = v_pool.tile([P, KT, E], BF, tag="V")
        pT_sb = pt_pool.tile([P, KT, QL], BF, tag="pT")

        cc_in_k = dram.tile([KHT, P, KH], F8, tag="cc_in_k")
        cc_out_k = dram.tile([2 * KHT, P, KH], F8, tag="cc_out_k")
        cc_in_v = dram.tile([KHT, P, E], BF, tag="cc_in_v")
        cc_out_v = dram.tile([2 * KHT, P, E], BF, tag="cc_out_v")

        def copy_out(dst, src, use_vector):
            if use_vector:
                nc.vector.tensor_copy(dst, src)
            else:
                nc.scalar.activation(dst, src, AFT.Copy)

        # ---- input DMAs: wk/kin split in halves, need-first order ----
        wk_t = [wpool.tile([P, E], BF, tag=f"w{dt}", name=f"wk{dt}")
                for dt in range(DT)]
        kin_t = [apool.tile([P, KH], BF, tag=f"a{dt}", name=f"kin{dt}")
                 for dt in range(DT)]
        for dt in range(DT):
            rows = slice(dt * P, (dt + 1) * P)
            nc.sync.dma_start(out=wk_t[dt][:, 0:512], in_=wkT[rows, 0:512])
            nc.scalar.dma_start(out=kin_t[dt][:, 0:512], in_=kinT[rows, 0:512])
        for dt in range(DT):
            rows = slice(dt * P, (dt + 1) * P)
            nc.sync.dma_start(out=wk_t[dt][:, 512:1024], in_=wkT[rows, 512:1024])
            nc.scalar.dma_start(out=kin_t[dt][:, 512:1024],
                                in_=kinT[rows, 512:1024])
        wv_t = [wpool.tile([P, E], BF, tag=f"w{dt}", name=f"wv{dt}")
                for dt in range(DT)]
        vin_t = [apool.tile([P, KH], BF, tag=f"a{dt}", name=f"vin{dt}")
                 for dt in range(DT)]
        for dt in range(DT):
            nc.sync.dma_start(out=wv_t[dt], in_=wvT[dt * P:(dt + 1) * P, :])
            nc.scalar.dma_start(out=vin_t[dt], in_=vinT[dt * P:(dt + 1) * P, :])

        # ---- Phase B': local K^T half -> kT_sb k-tiles 0..7 (fp8) ----
        # dt-outer sweeps over open half-groups so matmuls start as the
        # wk/kin half-DMAs land: (ets 0-3, c0) needs only the first halves.
        b_acc = {}
        for ets, c in (((0, 1, 2, 3), 0), ((4, 5, 6, 7), 0),
                       ((0, 1, 2, 3), 1), ((4, 5, 6, 7), 1)):
            for et in ets:
                b_acc[(et, c)] = ps.tile([P, 512], F32, tag="ps",
                                          name=f"bacc{et}_{c}")
            for dt in range(DT):
                for et in ets:
                    nc.tensor.matmul(
                        b_acc[(et, c)], wk_t[dt][:, et * P:(et + 1) * P],
                        kin_t[dt][:, c * 512:(c + 1) * 512],
                        start=(dt == 0), stop=(dt == DT - 1))
            for et in ets:
                copy_out(kT_sb[:, et, c * 512:(c + 1) * 512], b_acc[(et, c)],
                         (et + c) % 2 == 0)
        for et in range(ET):
            nc.scalar.dma_start(out=cc_in_k[et], in_=kT_sb[:, et, 0:KH])
        nc.gpsimd.collective_compute(
            "AllGather", mybir.AluOpType.bypass, replica_groups=RG,
            ins=[cc_in_k.opt()], outs=[cc_out_k.opt()])

        # ---- Phase C': local V half -> V_sb k-tiles 0..7 ----
        for kt in range(KHT):
            acc = [ps.tile([P, 512], F32, tag="ps", name=f"cacc{kt}_{c}")
                   for c in range(2)]
            for dt in range(DT):
                v_sl = vin_t[dt][:, kt * P:(kt + 1) * P]
                for c in range(2):
                    nc.tensor.matmul(
                        acc[c], v_sl, wv_t[dt][:, c * 512:(c + 1) * 512],
                        start=(dt == 0), stop=(dt == DT - 1))
            for c in range(2):
                copy_out(V_sb[:, kt, c * 512:(c + 1) * 512], acc[c], c == 0)
            nc.scalar.dma_start(out=cc_in_v[kt], in_=V_sb[:, kt, :])
        nc.gpsimd.collective_compute(
            "AllGather", mybir.AluOpType.bypass, replica_groups=RG,
            ins=[cc_in_v.opt()], outs=[cc_out_v.opt()])

        # ---- Phase A: Q^T = WqT.T @ qinT (overlaps the collectives) ----
        wq_t = [wpool.tile([P, E], BF, tag=f"w{dt}", name=f"wq{dt}")
                for dt in range(DT)]
        qin_t = [apool.tile([P, QL], BF, tag=f"a{dt}", name=f"qin{dt}")
                 for dt in range(DT)]
        for dt in range(DT):
            nc.sync.dma_start(out=wq_t[dt], in_=wqT[dt * P:(dt + 1) * P, :])
            nc.scalar.dma_start(out=qin_t[dt], in_=qinT[dt * P:(dt + 1) * P, :])
        for et in range(ET):
            acc = [ps.tile([P, 512], F32, tag="ps", name=f"aacc{et}_{c}")
                   for c in range(2)]
            for dt in range(DT):
                w_sl = wq_t[dt][:, et * P:(et + 1) * P]
                for c in range(2):
                    nc.tensor.matmul(
                        acc[c], w_sl, qin_t[dt][:, c * 512:(c + 1) * 512],
                        start=(dt == 0), stop=(dt == DT - 1))
            for c in range(2):
                copy_out(qT_sb[:, et, c * 512:(c + 1) * 512], acc[c], c == 0)

        # ---- unpack the PEER halves into k-tiles 8..15 ----
        # peer block start: 8 if I'm the even rank of the pair, else 0
        pid = nc.sync.partition_id()
        peer_start = 8 - (pid % 2) * 8
        src_k = cc_out_k[bass.ds(peer_start, KHT)].rearrange("t p c -> p t c")
        nc.sync.dma_start(out=kT_sb[:, :, KH:KL], in_=src_k)
        src_v = cc_out_v[bass.ds(peer_start, KHT)].rearrange("t p c -> p t c")
        nc.sync.dma_start(out=V_sb[:, KHT:KT, :], in_=src_v)

        # ---- Phase D: S^T (fp8 DoubleRow); P^T = exp(S^T/32768) ----
        # Q' = 32Q, K' = 32K  ->  S'/32768 = QK/32
        for kt in range(KT):
            acc = [ps.tile([P, 512], F32, tag="ps", name=f"dacc{kt}_{c}")
                   for c in range(2)]
            for t in range(ET // 2):
                k_sl = kT_sb[:, 2 * t:2 * t + 2, kt * P:(kt + 1) * P]
                for c in range(2):
                    nc.tensor.matmul(
                        acc[c], k_sl,
                        qT_sb[:, 2 * t:2 * t + 2, c * 512:(c + 1) * 512],
                        start=(t == 0), stop=(t == ET // 2 - 1),
                        perf_mode=DR)
            for c in range(2):
                nc.scalar.activation(pT_sb[:, kt, c * 512:(c + 1) * 512],
                                     acc[c], AFT.Exp, scale=1.0 / 32768.0)

        # ---- Phase E: O' = P^T.T @ V ; s = P^T.T @ 1 ; out = O'/s ----
        for qt in range(QT):
            acc = [ps.tile([P, 512], F32, tag="ps", name=f"eacc{qt}_{c}")
                   for c in range(2)]
            ssum = ps_s.tile([P, 1], F32, tag="ps_s")
            for kt in range(KT):
                p_sl = pT_sb[:, kt, qt * P:(qt + 1) * P]
                # ssum first: the softmax denominator completes two matmuls
                # before the accumulation does, hiding the reciprocal
                nc.tensor.matmul(ssum[:, 0:1], p_sl, ones_t[:, 0:1],
                                 start=(kt == 0), stop=(kt == KT - 1))
                for c in range(2):
                    nc.tensor.matmul(
                        acc[c], p_sl, V_sb[:, kt, c * 512:(c + 1) * 512],
                        start=(kt == 0), stop=(kt == KT - 1))
            r_t = r_pool.tile([P, 1], F32, tag="r")
            nc.vector.reciprocal(r_t, ssum[:, 0:1])
            o_t = o_pool.tile([P, E], F32, tag="o")
            for cc in range(4):
                sl = slice(cc * 256, (cc + 1) * 256)
                nc.scalar.activation(o_t[:, sl], acc[cc // 2][:, cc % 2 * 256:
                                                              cc % 2 * 256 + 256],
                                     AFT.Copy, scale=r_t[:, 0:1])
                eng = nc.sync if cc % 2 == 0 else nc.scalar
                eng.dma_start(out=out[qt * P:(qt + 1) * P, sl], in_=o_t[:, sl])

    nc.compile()
    return nc


def _get_nc():
    if "nc" not in _CACHE:
        _CACHE["nc"] = _build_nc()
    return _CACHE["nc"]


def make_in_maps(q, k, v, W_q, W_k, W_v):
    bf = ml_dtypes.bfloat16
    wqT = (np.asarray(W_q, dtype=np.float32) * 32.0).T.astype(bf)
    wkT = (np.asarray(W_k, dtype=np.float32) * 32.0).T.astype(bf)
    wvT = np.asarray(W_v, dtype=np.float32).T.astype(bf)
    in_maps = []
    for c in range(8):
        b, h = c // 2, c % 2
        sl = slice(h * 1024, (h + 1) * 1024)
        in_maps.append({
            "qinT": np.asarray(q[b, sl, :], dtype=np.float32).T.astype(bf),
            "kinT": np.asarray(k[b, sl, :], dtype=np.float32).T.astype(bf),
            "vinT": np.asarray(v[b, sl, :], dtype=np.float32).T.astype(bf),
            "wqT": wqT, "wkT": wkT, "wvT": wvT,
        })
    return in_maps


def kernel(**inputs):
    from concourse import bass_utils

    q = np.asarray(inputs["q_input"], dtype=np.float32)
    k = np.asarray(inputs["k_input"], dtype=np.float32)
    v = np.asarray(inputs["v_input"], dtype=np.float32)

    nc = _get_nc()
    in_maps = make_in_maps(q, k, v, inputs["W_q"], inputs["W_k"], inputs["W_v"])

    res = None
    for attempt in range(3):
        try:
            res = bass_utils.run_bass_kernel_spmd(nc, in_maps,
                                                  core_ids=list(range(8)))
            break
        except Exception:
            if attempt == 2:
                raise
    full = np.empty((4, 2048, 1024), dtype=np.float32)
    for c in range(8):
        b, h = c // 2, c % 2
        full[b, h * 1024:(h + 1) * 1024, :] = res.results[c]["out"]
    return full
